# revision 12
# baseline (speedup 1.0000x reference)
"""Trainium2 Bass kernel for nn_ConvNetFullOpti (BN + conv + heads + per-sample
Markowitz QP via PGD).

Strategy (validated against the reference in numpy, maxrel ~5e-4, l2rel ~1e-5):
- Pure data-parallel over 8 cores, 8192 samples each.
- The eigh/matrix-sqrt path is the identity on Q (eigenvalues >> 1e-8), so
  Q = cov directly.
- BN + conv + both linear layers fold into one affine map R^250 -> R^105 whose
  weights depend on the BN batch stats; stats are computed on-device by a small
  first kernel (per-core partial sums), combined on host (the 2KB all-reduce),
  folded into the weight matrix on host, then the main kernel runs.
- Projection onto {sum w = 1, |w| <= 1}: iteration 0 does 12 bisections + 3
  safeguarded Newton steps; subsequent PGD iterations warm-start tau and need
  only 2 safeguarded Newton steps to reproduce the reference's 60-iteration
  bisection to f32 accuracy.

Layout: samples on partitions. Per core: 64 blocks of 128 samples. L2 tiles are
(128 partitions, 64 blocks, ...) so every DVE op covers the whole shard.
"""

import numpy as np

N_CORES = 8
N_TOTAL = 65536
SHARD = N_TOTAL // N_CORES          # 8192
NB = SHARD // 128                   # 64 sample-blocks per core
LOOKBACK, N_ASSETS = 50, 5
NFEAT = 250
NOUT = 105                          # 100 cov-head + 5 ret-head
BN_EPS = 1e-5
PGD_ITERS = 150
NB0_BISECT = 12
NN0_NEWTON = 3
NN_WARM = 2

_cache = {}
TRACE = False           # set True (e.g. by test.py) to capture HW exec times
LAST_EXEC_NS = None     # sum over launches of max-core exec_time_ns when TRACE


def _build_stats_kernel():
    import concourse.bass as bass
    import concourse.tile as tile
    from concourse import bacc, mybir

    dt = mybir.dt
    OP = mybir.AluOpType

    nc = bacc.Bacc("TRN2", target_bir_lowering=False, debug=False)
    x_in = nc.dram_tensor("x", [SHARD, NFEAT], dt.float32, kind="ExternalInput").ap()
    stats_out = nc.dram_tensor("stats", [1, 2 * NFEAT], dt.float32, kind="ExternalOutput").ap()

    with tile.TileContext(nc) as tc:
        with tc.tile_pool(name="io", bufs=4) as io, \
             tc.tile_pool(name="accs", bufs=1) as accs, \
             tc.tile_pool(name="ps", bufs=2, space="PSUM") as ps:
            acc1 = accs.tile([128, NFEAT], dt.float32)
            acc2 = accs.tile([128, NFEAT], dt.float32)
            nc.vector.memset(acc1[:], 0.0)
            nc.vector.memset(acc2[:], 0.0)
            for b in range(NB):
                xb = io.tile([128, NFEAT], dt.float32)
                nc.sync.dma_start(xb[:], x_in[b * 128:(b + 1) * 128, :])
                sq = io.tile([128, NFEAT], dt.float32)
                nc.vector.tensor_tensor(sq[:], xb[:], xb[:], OP.mult)
                nc.vector.tensor_tensor(acc1[:], acc1[:], xb[:], OP.add)
                nc.vector.tensor_tensor(acc2[:], acc2[:], sq[:], OP.add)
            ones = accs.tile([128, 1], dt.float32)
            nc.vector.memset(ones[:], 1.0)
            pr1 = ps.tile([1, NFEAT], dt.float32)
            nc.tensor.matmul(pr1[:], ones[:], acc1[:], start=True, stop=True)
            pr2 = ps.tile([1, NFEAT], dt.float32)
            nc.tensor.matmul(pr2[:], ones[:], acc2[:], start=True, stop=True)
            out_sb = accs.tile([1, 2 * NFEAT], dt.float32)
            nc.vector.tensor_copy(out_sb[:, 0:NFEAT], pr1[:])
            nc.vector.tensor_copy(out_sb[:, NFEAT:2 * NFEAT], pr2[:])
            nc.sync.dma_start(stats_out, out_sb[:])
    nc.compile()
    return nc


def _build_main_kernel():
    import concourse.bass as bass
    import concourse.tile as tile
    from concourse import bacc, mybir, masks

    dt = mybir.dt
    OP = mybir.AluOpType
    AF = mybir.ActivationFunctionType
    AX = mybir.AxisListType
    f32 = dt.float32
    THRESH = float(np.float32(1.0 - 1e-7))

    nc = bacc.Bacc("TRN2", target_bir_lowering=False, debug=False)
    x_in = nc.dram_tensor("x", [SHARD, NFEAT], f32, kind="ExternalInput").ap()
    wt_in = nc.dram_tensor("wt", [NFEAT, NOUT], f32, kind="ExternalInput").ap()
    be_in = nc.dram_tensor("be", [1, NOUT], f32, kind="ExternalInput").ap()
    w_out = nc.dram_tensor("wout", [128, NB * 5], f32, kind="ExternalOutput").ap()

    with tile.TileContext(nc) as tc:
        import contextlib
        ctx = contextlib.ExitStack()
        with ctx:
            consts = ctx.enter_context(tc.tile_pool(name="consts", bufs=1))
            io = ctx.enter_context(tc.tile_pool(name="io", bufs=4))
            xtp = ctx.enter_context(tc.tile_pool(name="xtp", bufs=3))
            big = ctx.enter_context(tc.tile_pool(name="big", bufs=1))
            qp = ctx.enter_context(tc.tile_pool(name="qp", bufs=1))
            ps_t = ctx.enter_context(tc.tile_pool(name="ps_t", bufs=2, space="PSUM"))
            ps_o = ctx.enter_context(tc.tile_pool(name="ps_o", bufs=2, space="PSUM"))

            ident = consts.tile([128, 128], f32)
            masks.make_identity(nc, ident[:])
            ones_row = consts.tile([1, 128], f32)
            nc.vector.memset(ones_row[:], 1.0)
            w0 = consts.tile([125, NOUT], f32)
            nc.sync.dma_start(w0[:], wt_in[0:125, :])
            w1 = consts.tile([125, NOUT], f32)
            nc.sync.dma_start(w1[:], wt_in[125:250, :])
            be = consts.tile([1, NOUT], f32)
            nc.sync.dma_start(be[:], be_in)

            # big persistent L2 tensors; H/NHC are stored (block, asset, time) so
            # the per-(sample,asset) t-broadcast flattens to 3D access patterns
            # (the walrus verifier rejects >3D on TensorScalarPtr ops).
            H = big.tile([128, NB, 5, 20], f32)       # relu(cov-head), (a, t)
            RETS = big.tile([128, NB, 5], f32)
            NHC = big.tile([128, NB, 5, 20], f32)     # -(h - mean_t h), (a, t)
            Q4 = big.tile([128, NB, 5, 5], f32)

            # ---------------- feedforward ----------------
            for b in range(NB):
                xb = io.tile([128, NFEAT], f32)
                nc.sync.dma_start(xb[:], x_in[b * 128:(b + 1) * 128, :])
                xt0 = xtp.tile([125, 128], f32, tag="xt0")
                xt1 = xtp.tile([125, 128], f32, tag="xt1")
                pst0 = ps_t.tile([125, 128], f32, tag="pst0")
                nc.tensor.transpose(pst0[:], xb[:, 0:125], ident[:])
                nc.vector.tensor_copy(xt0[:], pst0[:])
                pst1 = ps_t.tile([125, 128], f32, tag="pst1")
                nc.tensor.transpose(pst1[:], xb[:, 125:250], ident[:])
                nc.scalar.copy(xt1[:], pst1[:])
                po = ps_o.tile([128, NOUT], f32)
                nc.tensor.matmul(po[:], xt0[:], w0[:], start=True, stop=False)
                nc.tensor.matmul(po[:], xt1[:], w1[:], start=False, stop=False)
                nc.tensor.matmul(po[:], ones_row[:], be[:], start=False, stop=True)
                # po columns are (t, a)-ordered; H is (a, t): write via transposed AP
                nc.scalar.activation(
                    H[:, b, :, :].transpose([0, 2, 1]), po[:, 0:100], AF.Relu)
                nc.scalar.activation(RETS[:, b, :], po[:, 100:105], AF.Tanh)

            # ---------------- covariance -> Q ----------------
            HM = qp.tile([128, NB, 5], f32)
            h3 = H[:].rearrange("p b a t -> p (b a) t")
            nc.vector.tensor_reduce(
                HM[:].rearrange("p b a -> p (b a)"), h3, axis=AX.X, op=OP.add)
            # NHC = HM/20 - H  (negated hc; sign cancels in cov)
            hm_b = HM[:].rearrange("p b a -> p (b a)").unsqueeze(2).broadcast_to(
                [128, NB * 5, 20])
            nc.vector.scalar_tensor_tensor(
                NHC[:].rearrange("p b a t -> p (b a) t"), hm_b, 1.0 / 20.0, h3,
                OP.mult, OP.subtract)
            PR20 = qp.tile([128, NB, 20], f32)
            RED = qp.tile([128, NB], f32)
            for i in range(5):
                for j in range(i, 5):
                    nc.vector.tensor_tensor(PR20[:], NHC[:, :, i, :], NHC[:, :, j, :], OP.mult)
                    nc.vector.tensor_reduce(RED[:], PR20[:], axis=AX.X, op=OP.add)
                    nc.vector.tensor_scalar(Q4[:, :, i, j], RED[:], 1.0 / 19.0, None, OP.mult)
                    if i != j:
                        nc.vector.tensor_copy(Q4[:, :, j, i], Q4[:, :, i, j])

            TR = qp.tile([128, NB], f32)
            T1S = qp.tile([128, NB], f32)
            nc.vector.tensor_tensor(TR[:], Q4[:, :, 0, 0], Q4[:, :, 1, 1], OP.add)
            nc.vector.tensor_tensor(TR[:], TR[:], Q4[:, :, 2, 2], OP.add)
            nc.vector.tensor_tensor(TR[:], TR[:], Q4[:, :, 3, 3], OP.add)
            nc.vector.tensor_tensor(TR[:], TR[:], Q4[:, :, 4, 4], OP.add)
            ETA = qp.tile([128, NB], f32)
            nc.vector.tensor_scalar(T1S[:], TR[:], 2.0, 2e-6, OP.mult, OP.add)
            nc.vector.reciprocal(ETA[:], T1S[:])    # eta = 0.5/(tr + 1e-6)

            # ---------------- QP state ----------------
            W = qp.tile([128, NB, 5], f32)
            nc.vector.memset(W[:], 0.2)
            V = qp.tile([128, NB, 5], f32)
            G = qp.tile([128, NB, 5], f32)
            TV = qp.tile([128, NB, 5], f32)
            QW = qp.tile([128, NB, 5], f32)
            T = qp.tile([128, NB, 5], f32)
            C = qp.tile([128, NB, 5], f32)
            AEQ = qp.tile([128, NB, 5], f32)
            NT = qp.tile([128, NB, 5], f32)
            AB = qp.tile([128, NB, 5], f32)
            A2 = qp.tile([128, NB, 5], f32)
            T5 = qp.tile([128, NB, 5], f32)
            TAU = qp.tile([128, NB], f32)
            TAU2 = qp.tile([128, NB], f32)
            LOa = qp.tile([128, NB], f32)
            LOb = qp.tile([128, NB], f32)
            HIa = qp.tile([128, NB], f32)
            HIb = qp.tile([128, NB], f32)
            VMIN = qp.tile([128, NB], f32)
            VMAX = qp.tile([128, NB], f32)
            S = qp.tile([128, NB], f32)
            NA = qp.tile([128, NB], f32)
            PRED = qp.tile([128, NB], dt.int32)   # copy_predicated needs int mask
            NAC = qp.tile([128, NB], f32)
            RNA = qp.tile([128, NB], f32)
            STEP = qp.tile([128, NB], f32)
            CAND = qp.tile([128, NB], f32)
            TT1 = qp.tile([128, NB], f32)
            S2 = qp.tile([128, NB], f32)
            D = qp.tile([128, NB], f32)
            CORR = qp.tile([128, NB], f32)

            def flat(t):
                return t[:].rearrange("p b a -> p (b a)")

            def bcast(t):
                return t[:].unsqueeze(2).broadcast_to([128, NB, 5])

            def eval_t_c_s(tau_t):
                # t = v - tau ; c = clip(t) ; s = sum_a c
                nc.vector.scalar_tensor_tensor(T[:], V[:], 1.0, bcast(tau_t), OP.mult, OP.subtract)
                nc.vector.tensor_scalar(flat(C), flat(T), -1.0, 1.0, OP.max, OP.min)
                nc.vector.tensor_reduce(S[:], C[:], axis=AX.X, op=OP.add)

            def bracket_update(lo, hi, lo2, hi2, tau_t):
                nc.vector.tensor_scalar(PRED[:], S[:], 1.0, None, OP.is_gt)
                nc.vector.select(lo2[:], PRED[:], tau_t[:], lo[:])
                nc.vector.select(hi2[:], PRED[:], hi[:], tau_t[:])

            PR5 = qp.tile([128, NB, 5], f32)
            for it in range(PGD_ITERS):
                # grad + step: Qw_i = sum_j Q[i,j] w_j, per output asset i
                for i in range(5):
                    nc.vector.tensor_tensor(PR5[:], Q4[:, :, i, :], W[:], OP.mult)
                    nc.vector.tensor_reduce(QW[:, :, i], PR5[:], axis=AX.X, op=OP.add)
                nc.vector.scalar_tensor_tensor(G[:], QW[:], 2.0, RETS[:], OP.mult, OP.subtract)
                nc.vector.tensor_tensor(TV[:], G[:], bcast(ETA), OP.mult)
                nc.vector.tensor_tensor(V[:], W[:], TV[:], OP.subtract)
                # bracket
                nc.vector.tensor_reduce(VMIN[:], V[:], axis=AX.X, op=OP.min)
                nc.vector.tensor_reduce(VMAX[:], V[:], axis=AX.X, op=OP.max)
                lo, hi, lo2, hi2 = LOa, HIa, LOb, HIb
                nc.vector.tensor_scalar(lo[:], VMIN[:], -2.0, None, OP.add)
                nc.vector.tensor_scalar(hi[:], VMAX[:], 2.0, None, OP.add)
                tau, tau2 = TAU, TAU2
                if it == 0:
                    nc.vector.tensor_tensor(TT1[:], lo[:], hi[:], OP.add)
                    nc.vector.tensor_scalar(tau[:], TT1[:], 0.5, None, OP.mult)
                    for _ in range(NB0_BISECT):
                        eval_t_c_s(tau)
                        bracket_update(lo, hi, lo2, hi2, tau)
                        lo, lo2 = lo2, lo
                        hi, hi2 = hi2, hi
                        nc.vector.tensor_tensor(TT1[:], lo[:], hi[:], OP.add)
                        nc.vector.tensor_scalar(tau[:], TT1[:], 0.5, None, OP.mult)
                    n_newton = NN0_NEWTON
                else:
                    # clamp warm tau into bracket
                    nc.vector.tensor_tensor(TT1[:], tau[:], lo[:], OP.max)
                    nc.vector.tensor_tensor(tau2[:], TT1[:], hi[:], OP.min)
                    tau, tau2 = tau2, tau
                    n_newton = NN_WARM
                for _ in range(n_newton):
                    eval_t_c_s(tau)
                    nc.vector.tensor_tensor(AEQ[:], C[:], T[:], OP.is_equal)
                    nc.vector.tensor_reduce(NA[:], AEQ[:], axis=AX.X, op=OP.add)
                    bracket_update(lo, hi, lo2, hi2, tau)
                    lo, lo2 = lo2, lo
                    hi, hi2 = hi2, hi
                    nc.vector.tensor_scalar(NAC[:], NA[:], 1.0, None, OP.max)
                    nc.vector.reciprocal(RNA[:], NAC[:])
                    nc.vector.scalar_tensor_tensor(STEP[:], S[:], -1.0, RNA[:], OP.add, OP.mult)
                    nc.vector.tensor_tensor(CAND[:], tau[:], STEP[:], OP.add)
                    nc.vector.tensor_tensor(TT1[:], CAND[:], lo[:], OP.max)
                    nc.vector.tensor_tensor(tau2[:], TT1[:], hi[:], OP.min)
                    tau, tau2 = tau2, tau
                # keep final tau in TAU for warm start next iteration
                if tau is not TAU:
                    TAU, TAU2 = TAU2, TAU
                # finalize: w = w0 + a*(1 - sum w0)/na
                eval_t_c_s(tau)   # C = w0, S = sum w0
                nc.vector.tensor_scalar(flat(NT), flat(T), -1.0, None, OP.mult)
                nc.vector.tensor_tensor(AB[:], T[:], NT[:], OP.max)
                nc.vector.tensor_scalar(flat(A2), flat(AB), THRESH, None, OP.is_lt)
                nc.vector.tensor_reduce(NA[:], A2[:], axis=AX.X, op=OP.add)
                nc.vector.tensor_scalar(NAC[:], NA[:], 1.0, None, OP.max)
                nc.vector.reciprocal(RNA[:], NAC[:])
                nc.vector.tensor_scalar(D[:], S[:], -1.0, 1.0, OP.mult, OP.add)
                nc.vector.tensor_tensor(CORR[:], D[:], RNA[:], OP.mult)
                nc.vector.scalar_tensor_tensor(T5[:], A2[:], 1.0, bcast(CORR), OP.mult, OP.mult)
                nc.vector.tensor_tensor(W[:], C[:], T5[:], OP.add)

            nc.sync.dma_start(w_out, flat(W))
    nc.compile()
    return nc


def _fold_weights(stats_sum, bn_gamma, bn_beta, conv_w, conv_b, lin_w, lin_b,
                  linc_w, linc_b):
    """Combine global BN stats with the small weights into W_effT (250,105), b_eff."""
    s1 = stats_sum[0]
    s2 = stats_sum[1]
    mu = (s1 / N_TOTAL).astype(np.float32)
    var = (s2 / N_TOTAL - mu * mu).astype(np.float32)
    scale = (bn_gamma / np.sqrt(var + np.float32(BN_EPS))).astype(np.float32)
    shift = (bn_beta - mu * scale).astype(np.float32)

    def conv_mat(c):
        C = np.zeros((120, NFEAT), np.float32)
        for t in range(24):
            for a in range(5):
                for k in range(3):
                    C[5 * t + a, 5 * (2 * t + k) + a] = conv_w[c, 0, k, 0]
        return C

    M = np.vstack([linc_w @ conv_mat(1), lin_w @ conv_mat(0)]).astype(np.float32)
    b0 = np.concatenate([
        linc_b + linc_w.sum(1) * conv_b[1],
        lin_b + lin_w.sum(1) * conv_b[0],
    ]).astype(np.float32)
    W_eff = (M * scale[None, :]).astype(np.float32)
    b_eff = (M @ shift + b0).astype(np.float32)
    return np.ascontiguousarray(W_eff.T), b_eff.reshape(1, NOUT)


def kernel(x, bn_gamma, bn_beta, conv_w, conv_b, lin_w, lin_b, linc_w, linc_b):
    from concourse.bass_utils import run_bass_kernel_spmd

    x = np.ascontiguousarray(np.asarray(x, np.float32).reshape(N_TOTAL, NFEAT))
    bn_gamma = np.asarray(bn_gamma, np.float32)
    bn_beta = np.asarray(bn_beta, np.float32)
    conv_w = np.asarray(conv_w, np.float32)
    conv_b = np.asarray(conv_b, np.float32)
    lin_w = np.asarray(lin_w, np.float32)
    lin_b = np.asarray(lin_b, np.float32)
    linc_w = np.asarray(linc_w, np.float32)
    linc_b = np.asarray(linc_b, np.float32)

    core_ids = list(range(N_CORES))
    shards = [np.ascontiguousarray(x[c * SHARD:(c + 1) * SHARD]) for c in core_ids]

    global LAST_EXEC_NS
    exec_ns = 0
    if "stats" not in _cache:
        _cache["stats"] = _build_stats_kernel()
    nc1 = _cache["stats"]
    res1 = run_bass_kernel_spmd(nc1, [{"x": s} for s in shards], core_ids, trace=TRACE)
    if res1.exec_time_ns:
        exec_ns += res1.exec_time_ns
    stats_sum = np.sum(
        [r["stats"].reshape(2, NFEAT).astype(np.float64) for r in res1.results], axis=0)

    wt, be = _fold_weights(stats_sum.astype(np.float32), bn_gamma, bn_beta, conv_w,
                           conv_b, lin_w, lin_b, linc_w, linc_b)

    if "main" not in _cache:
        _cache["main"] = _build_main_kernel()
    nc2 = _cache["main"]
    in_maps = [{"x": s, "wt": wt, "be": be} for s in shards]
    res2 = run_bass_kernel_spmd(nc2, in_maps, core_ids, trace=TRACE)
    if res2.exec_time_ns:
        exec_ns += res2.exec_time_ns
    LAST_EXEC_NS = exec_ns if exec_ns else None

    outs = []
    for r in res2.results:
        wo = r["wout"].reshape(128, NB, 5).transpose(1, 0, 2).reshape(SHARD, 5)
        outs.append(wo)
    return np.ascontiguousarray(np.concatenate(outs, axis=0))


if __name__ == "__main__":
    pass


# revision 13
# speedup vs baseline: 1.5930x; 1.5930x over previous
"""Trainium2 Bass kernel for nn_ConvNetFullOpti (BN + conv + heads + per-sample
Markowitz QP via PGD).

Strategy (validated against the reference in numpy, maxrel ~5e-4, l2rel ~1e-5):
- Pure data-parallel over 8 cores, 8192 samples each.
- The eigh/matrix-sqrt path is the identity on Q (eigenvalues >> 1e-8), so
  Q = cov directly.
- BN + conv + both linear layers fold into one affine map R^250 -> R^105 whose
  weights depend on the BN batch stats; stats are computed on-device by a small
  first kernel (per-core partial sums), combined on host (the 2KB all-reduce),
  folded into the weight matrix on host, then the main kernel runs.
- Projection onto {sum w = 1, |w| <= 1}: iteration 0 does 12 bisections + 3
  safeguarded Newton steps; subsequent PGD iterations warm-start tau and need
  only 2 safeguarded Newton steps to reproduce the reference's 60-iteration
  bisection to f32 accuracy.

Layout: samples on partitions. Per core: 64 blocks of 128 samples. L2 tiles are
(128 partitions, 64 blocks, ...) so every DVE op covers the whole shard.
"""

import numpy as np

N_CORES = 8
N_TOTAL = 65536
SHARD = N_TOTAL // N_CORES          # 8192
NB = SHARD // 128                   # 64 sample-blocks per core
LOOKBACK, N_ASSETS = 50, 5
NFEAT = 250
NOUT = 105                          # 100 cov-head + 5 ret-head
BN_EPS = 1e-5
PGD_ITERS = 150
NB0_BISECT = 12
NN0_NEWTON = 3
NN_WARM = 2

_cache = {}
TRACE = False           # set True (e.g. by test.py) to capture HW exec times
LAST_EXEC_NS = None     # sum over launches of max-core exec_time_ns when TRACE


def _build_stats_kernel():
    import concourse.bass as bass
    import concourse.tile as tile
    from concourse import bacc, mybir

    dt = mybir.dt
    OP = mybir.AluOpType

    nc = bacc.Bacc("TRN2", target_bir_lowering=False, debug=False)
    x_in = nc.dram_tensor("x", [SHARD, NFEAT], dt.float32, kind="ExternalInput").ap()
    stats_out = nc.dram_tensor("stats", [1, 2 * NFEAT], dt.float32, kind="ExternalOutput").ap()

    with tile.TileContext(nc) as tc:
        with tc.tile_pool(name="io", bufs=4) as io, \
             tc.tile_pool(name="accs", bufs=1) as accs, \
             tc.tile_pool(name="ps", bufs=2, space="PSUM") as ps:
            acc1 = accs.tile([128, NFEAT], dt.float32)
            acc2 = accs.tile([128, NFEAT], dt.float32)
            nc.vector.memset(acc1[:], 0.0)
            nc.vector.memset(acc2[:], 0.0)
            for b in range(NB):
                xb = io.tile([128, NFEAT], dt.float32)
                nc.sync.dma_start(xb[:], x_in[b * 128:(b + 1) * 128, :])
                sq = io.tile([128, NFEAT], dt.float32)
                nc.vector.tensor_tensor(sq[:], xb[:], xb[:], OP.mult)
                nc.vector.tensor_tensor(acc1[:], acc1[:], xb[:], OP.add)
                nc.vector.tensor_tensor(acc2[:], acc2[:], sq[:], OP.add)
            ones = accs.tile([128, 1], dt.float32)
            nc.vector.memset(ones[:], 1.0)
            pr1 = ps.tile([1, NFEAT], dt.float32)
            nc.tensor.matmul(pr1[:], ones[:], acc1[:], start=True, stop=True)
            pr2 = ps.tile([1, NFEAT], dt.float32)
            nc.tensor.matmul(pr2[:], ones[:], acc2[:], start=True, stop=True)
            out_sb = accs.tile([1, 2 * NFEAT], dt.float32)
            nc.vector.tensor_copy(out_sb[:, 0:NFEAT], pr1[:])
            nc.vector.tensor_copy(out_sb[:, NFEAT:2 * NFEAT], pr2[:])
            nc.sync.dma_start(stats_out, out_sb[:])
    nc.compile()
    return nc


def _build_main_kernel():
    import concourse.bass as bass
    import concourse.tile as tile
    from concourse import bacc, mybir, masks

    dt = mybir.dt
    OP = mybir.AluOpType
    AF = mybir.ActivationFunctionType
    AX = mybir.AxisListType
    f32 = dt.float32
    THRESH = float(np.float32(1.0 - 1e-7))

    nc = bacc.Bacc("TRN2", target_bir_lowering=False, debug=False)
    x_in = nc.dram_tensor("x", [SHARD, NFEAT], f32, kind="ExternalInput").ap()
    wt_in = nc.dram_tensor("wt", [NFEAT, NOUT], f32, kind="ExternalInput").ap()
    be_in = nc.dram_tensor("be", [1, NOUT], f32, kind="ExternalInput").ap()
    w_out = nc.dram_tensor("wout", [128, NB * 5], f32, kind="ExternalOutput").ap()

    with tile.TileContext(nc) as tc:
        import contextlib
        ctx = contextlib.ExitStack()
        with ctx:
            consts = ctx.enter_context(tc.tile_pool(name="consts", bufs=1))
            io = ctx.enter_context(tc.tile_pool(name="io", bufs=4))
            xtp = ctx.enter_context(tc.tile_pool(name="xtp", bufs=3))
            big = ctx.enter_context(tc.tile_pool(name="big", bufs=1))
            qp = ctx.enter_context(tc.tile_pool(name="qp", bufs=1))
            ps_t = ctx.enter_context(tc.tile_pool(name="ps_t", bufs=2, space="PSUM"))
            ps_o = ctx.enter_context(tc.tile_pool(name="ps_o", bufs=2, space="PSUM"))

            ident = consts.tile([128, 128], f32)
            masks.make_identity(nc, ident[:])
            ones_row = consts.tile([1, 128], f32)
            nc.vector.memset(ones_row[:], 1.0)
            w0 = consts.tile([125, NOUT], f32)
            nc.sync.dma_start(w0[:], wt_in[0:125, :])
            w1 = consts.tile([125, NOUT], f32)
            nc.sync.dma_start(w1[:], wt_in[125:250, :])
            be = consts.tile([1, NOUT], f32)
            nc.sync.dma_start(be[:], be_in)

            # big persistent L2 tensors; H/NHC are stored (block, asset, time) so
            # the per-(sample,asset) t-broadcast flattens to 3D access patterns
            # (the walrus verifier rejects >3D on TensorScalarPtr ops).
            H = big.tile([128, NB, 5, 20], f32)       # relu(cov-head), (a, t)
            RETS = big.tile([128, NB, 5], f32)
            NHC = big.tile([128, NB, 5, 20], f32)     # -(h - mean_t h), (a, t)
            Q4 = big.tile([128, NB, 5, 5], f32)

            # ---------------- feedforward ----------------
            for b in range(NB):
                xb = io.tile([128, NFEAT], f32)
                nc.sync.dma_start(xb[:], x_in[b * 128:(b + 1) * 128, :])
                xt0 = xtp.tile([125, 128], f32, tag="xt0")
                xt1 = xtp.tile([125, 128], f32, tag="xt1")
                pst0 = ps_t.tile([125, 128], f32, tag="pst0")
                nc.tensor.transpose(pst0[:], xb[:, 0:125], ident[:])
                nc.vector.tensor_copy(xt0[:], pst0[:])
                pst1 = ps_t.tile([125, 128], f32, tag="pst1")
                nc.tensor.transpose(pst1[:], xb[:, 125:250], ident[:])
                nc.scalar.copy(xt1[:], pst1[:])
                po = ps_o.tile([128, NOUT], f32)
                nc.tensor.matmul(po[:], xt0[:], w0[:], start=True, stop=False)
                nc.tensor.matmul(po[:], xt1[:], w1[:], start=False, stop=False)
                nc.tensor.matmul(po[:], ones_row[:], be[:], start=False, stop=True)
                # po columns are (t, a)-ordered; H is (a, t): write via transposed AP
                nc.scalar.activation(
                    H[:, b, :, :].transpose([0, 2, 1]), po[:, 0:100], AF.Relu)
                nc.scalar.activation(RETS[:, b, :], po[:, 100:105], AF.Tanh)

            # ---------------- covariance -> Q ----------------
            HM = qp.tile([128, NB, 5], f32)
            h3 = H[:].rearrange("p b a t -> p (b a) t")
            nc.vector.tensor_reduce(
                HM[:].rearrange("p b a -> p (b a)"), h3, axis=AX.X, op=OP.add)
            # NHC = HM/20 - H  (negated hc; sign cancels in cov)
            hm_b = HM[:].rearrange("p b a -> p (b a)").unsqueeze(2).broadcast_to(
                [128, NB * 5, 20])
            nc.vector.scalar_tensor_tensor(
                NHC[:].rearrange("p b a t -> p (b a) t"), hm_b, 1.0 / 20.0, h3,
                OP.mult, OP.subtract)
            PR20 = qp.tile([128, NB, 20], f32)
            RED = qp.tile([128, NB], f32)
            for i in range(5):
                for j in range(i, 5):
                    nc.vector.tensor_tensor(PR20[:], NHC[:, :, i, :], NHC[:, :, j, :], OP.mult)
                    nc.vector.tensor_reduce(RED[:], PR20[:], axis=AX.X, op=OP.add)
                    nc.vector.tensor_scalar(Q4[:, :, i, j], RED[:], 1.0 / 19.0, None, OP.mult)
                    if i != j:
                        nc.vector.tensor_copy(Q4[:, :, j, i], Q4[:, :, i, j])

            TR = qp.tile([128, NB], f32)
            T1S = qp.tile([128, NB], f32)
            nc.vector.tensor_tensor(TR[:], Q4[:, :, 0, 0], Q4[:, :, 1, 1], OP.add)
            nc.vector.tensor_tensor(TR[:], TR[:], Q4[:, :, 2, 2], OP.add)
            nc.vector.tensor_tensor(TR[:], TR[:], Q4[:, :, 3, 3], OP.add)
            nc.vector.tensor_tensor(TR[:], TR[:], Q4[:, :, 4, 4], OP.add)
            ETA = qp.tile([128, NB], f32)
            nc.vector.tensor_scalar(T1S[:], TR[:], 2.0, 2e-6, OP.mult, OP.add)
            nc.vector.reciprocal(ETA[:], T1S[:])    # eta = 0.5/(tr + 1e-6)

            # ---------------- QP state ----------------
            W = qp.tile([128, NB, 5], f32)
            nc.vector.memset(W[:], 0.2)
            V = qp.tile([128, NB, 5], f32)
            G = qp.tile([128, NB, 5], f32)
            TV = qp.tile([128, NB, 5], f32)
            QW = qp.tile([128, NB, 5], f32)
            T = qp.tile([128, NB, 5], f32)
            C = qp.tile([128, NB, 5], f32)
            AEQ = qp.tile([128, NB, 5], f32)
            NT = qp.tile([128, NB, 5], f32)
            AB = qp.tile([128, NB, 5], f32)
            A2 = qp.tile([128, NB, 5], f32)
            T5 = qp.tile([128, NB, 5], f32)
            TAU = qp.tile([128, NB], f32)
            TAU2 = qp.tile([128, NB], f32)
            LOa = qp.tile([128, NB], f32)
            LOb = qp.tile([128, NB], f32)
            HIa = qp.tile([128, NB], f32)
            HIb = qp.tile([128, NB], f32)
            VMIN = qp.tile([128, NB], f32)
            VMAX = qp.tile([128, NB], f32)
            S = qp.tile([128, NB], f32)
            NA = qp.tile([128, NB], f32)
            PRED = qp.tile([128, NB], dt.int32)   # copy_predicated needs int mask
            NAC = qp.tile([128, NB], f32)
            RNA = qp.tile([128, NB], f32)
            STEP = qp.tile([128, NB], f32)
            CAND = qp.tile([128, NB], f32)
            TT1 = qp.tile([128, NB], f32)
            S2 = qp.tile([128, NB], f32)
            D = qp.tile([128, NB], f32)
            CORR = qp.tile([128, NB], f32)

            def flat(t):
                return t[:].rearrange("p b a -> p (b a)")

            def bcast(t):
                return t[:].unsqueeze(2).broadcast_to([128, NB, 5])

            def eval_t_c_s(tau_t):
                # t = v - tau ; c = clip(t) ; s = sum_a c
                nc.vector.scalar_tensor_tensor(T[:], V[:], 1.0, bcast(tau_t), OP.mult, OP.subtract)
                nc.vector.tensor_scalar(flat(C), flat(T), -1.0, 1.0, OP.max, OP.min)
                nc.vector.tensor_reduce(S[:], C[:], axis=AX.X, op=OP.add)

            def bracket_update(lo, hi, lo2, hi2, tau_t):
                nc.vector.tensor_scalar(PRED[:], S[:], 1.0, None, OP.is_gt)
                nc.vector.select(lo2[:], PRED[:], tau_t[:], lo[:])
                nc.vector.select(hi2[:], PRED[:], hi[:], tau_t[:])

            PR5 = qp.tile([128, NB, 5], f32)
            for it in range(PGD_ITERS):
                # grad + step: Qw_i = sum_j Q[i,j] w_j, per output asset i
                for i in range(5):
                    nc.vector.tensor_tensor(PR5[:], Q4[:, :, i, :], W[:], OP.mult)
                    nc.vector.tensor_reduce(QW[:, :, i], PR5[:], axis=AX.X, op=OP.add)
                nc.vector.scalar_tensor_tensor(G[:], QW[:], 2.0, RETS[:], OP.mult, OP.subtract)
                nc.vector.tensor_tensor(TV[:], G[:], bcast(ETA), OP.mult)
                nc.vector.tensor_tensor(V[:], W[:], TV[:], OP.subtract)
                nc.vector.tensor_reduce(VMIN[:], V[:], axis=AX.X, op=OP.min)
                nc.vector.tensor_reduce(VMAX[:], V[:], axis=AX.X, op=OP.max)
                tau, tau2 = TAU, TAU2
                if it == 0:
                    lo, hi, lo2, hi2 = LOa, HIa, LOb, HIb
                    nc.vector.tensor_scalar(lo[:], VMIN[:], -2.0, None, OP.add)
                    nc.vector.tensor_scalar(hi[:], VMAX[:], 2.0, None, OP.add)
                    nc.vector.tensor_tensor(TT1[:], lo[:], hi[:], OP.add)
                    nc.vector.tensor_scalar(tau[:], TT1[:], 0.5, None, OP.mult)
                    for _ in range(NB0_BISECT):
                        eval_t_c_s(tau)
                        bracket_update(lo, hi, lo2, hi2, tau)
                        lo, lo2 = lo2, lo
                        hi, hi2 = hi2, hi
                        nc.vector.tensor_tensor(TT1[:], lo[:], hi[:], OP.add)
                        nc.vector.tensor_scalar(tau[:], TT1[:], 0.5, None, OP.mult)
                    for _ in range(NN0_NEWTON):
                        eval_t_c_s(tau)
                        nc.vector.tensor_tensor(AEQ[:], C[:], T[:], OP.is_equal)
                        nc.vector.tensor_reduce(NA[:], AEQ[:], axis=AX.X, op=OP.add)
                        bracket_update(lo, hi, lo2, hi2, tau)
                        lo, lo2 = lo2, lo
                        hi, hi2 = hi2, hi
                        nc.vector.tensor_scalar(NAC[:], NA[:], 1.0, None, OP.max)
                        nc.vector.reciprocal(RNA[:], NAC[:])
                        nc.vector.scalar_tensor_tensor(STEP[:], S[:], -1.0, RNA[:], OP.add, OP.mult)
                        nc.vector.tensor_tensor(CAND[:], tau[:], STEP[:], OP.add)
                        nc.vector.tensor_tensor(TT1[:], CAND[:], lo[:], OP.max)
                        nc.vector.tensor_tensor(tau2[:], TT1[:], hi[:], OP.min)
                        tau, tau2 = tau2, tau
                else:
                    # clamp warm tau into [vmin-2, vmax+2] via fused stt ops
                    nc.vector.scalar_tensor_tensor(TT1[:], VMIN[:], -2.0, tau[:], OP.add, OP.max)
                    nc.vector.scalar_tensor_tensor(tau2[:], VMAX[:], 2.0, TT1[:], OP.add, OP.min)
                    tau, tau2 = tau2, tau
                # fused eval + finalize + tau update (shares one reciprocal):
                #   w  = c + aeq*(1-s)/max(na,1)
                #   tau' = clamp(tau - (1-s)/max(na,1), vmin-2, vmax+2)
                eval_t_c_s(tau)   # T, C = w0, S = sum w0
                nc.vector.tensor_tensor(AEQ[:], C[:], T[:], OP.is_equal)
                nc.vector.tensor_reduce(NA[:], AEQ[:], axis=AX.X, op=OP.add)
                nc.vector.tensor_scalar(NAC[:], NA[:], 1.0, None, OP.max)
                nc.vector.reciprocal(RNA[:], NAC[:])
                nc.vector.tensor_scalar(D[:], S[:], -1.0, 1.0, OP.mult, OP.add)
                nc.vector.tensor_tensor(CORR[:], D[:], RNA[:], OP.mult)
                nc.vector.scalar_tensor_tensor(T5[:], AEQ[:], 1.0, bcast(CORR), OP.mult, OP.mult)
                nc.vector.tensor_tensor(W[:], C[:], T5[:], OP.add)
                nc.vector.tensor_tensor(CAND[:], tau[:], CORR[:], OP.subtract)
                nc.vector.scalar_tensor_tensor(TT1[:], VMIN[:], -2.0, CAND[:], OP.add, OP.max)
                nc.vector.scalar_tensor_tensor(tau2[:], VMAX[:], 2.0, TT1[:], OP.add, OP.min)
                tau, tau2 = tau2, tau
                if tau is not TAU:
                    TAU, TAU2 = TAU2, TAU

            nc.sync.dma_start(w_out, flat(W))
    nc.compile()
    return nc


def _fold_weights(stats_sum, bn_gamma, bn_beta, conv_w, conv_b, lin_w, lin_b,
                  linc_w, linc_b):
    """Combine global BN stats with the small weights into W_effT (250,105), b_eff."""
    s1 = stats_sum[0]
    s2 = stats_sum[1]
    mu = (s1 / N_TOTAL).astype(np.float32)
    var = (s2 / N_TOTAL - mu * mu).astype(np.float32)
    scale = (bn_gamma / np.sqrt(var + np.float32(BN_EPS))).astype(np.float32)
    shift = (bn_beta - mu * scale).astype(np.float32)

    def conv_mat(c):
        C = np.zeros((120, NFEAT), np.float32)
        for t in range(24):
            for a in range(5):
                for k in range(3):
                    C[5 * t + a, 5 * (2 * t + k) + a] = conv_w[c, 0, k, 0]
        return C

    M = np.vstack([linc_w @ conv_mat(1), lin_w @ conv_mat(0)]).astype(np.float32)
    b0 = np.concatenate([
        linc_b + linc_w.sum(1) * conv_b[1],
        lin_b + lin_w.sum(1) * conv_b[0],
    ]).astype(np.float32)
    W_eff = (M * scale[None, :]).astype(np.float32)
    b_eff = (M @ shift + b0).astype(np.float32)
    return np.ascontiguousarray(W_eff.T), b_eff.reshape(1, NOUT)


def kernel(x, bn_gamma, bn_beta, conv_w, conv_b, lin_w, lin_b, linc_w, linc_b):
    from concourse.bass_utils import run_bass_kernel_spmd

    x = np.ascontiguousarray(np.asarray(x, np.float32).reshape(N_TOTAL, NFEAT))
    bn_gamma = np.asarray(bn_gamma, np.float32)
    bn_beta = np.asarray(bn_beta, np.float32)
    conv_w = np.asarray(conv_w, np.float32)
    conv_b = np.asarray(conv_b, np.float32)
    lin_w = np.asarray(lin_w, np.float32)
    lin_b = np.asarray(lin_b, np.float32)
    linc_w = np.asarray(linc_w, np.float32)
    linc_b = np.asarray(linc_b, np.float32)

    core_ids = list(range(N_CORES))
    shards = [np.ascontiguousarray(x[c * SHARD:(c + 1) * SHARD]) for c in core_ids]

    global LAST_EXEC_NS
    exec_ns = 0
    if "stats" not in _cache:
        _cache["stats"] = _build_stats_kernel()
    nc1 = _cache["stats"]
    res1 = run_bass_kernel_spmd(nc1, [{"x": s} for s in shards], core_ids, trace=TRACE)
    if res1.exec_time_ns:
        exec_ns += res1.exec_time_ns
    stats_sum = np.sum(
        [r["stats"].reshape(2, NFEAT).astype(np.float64) for r in res1.results], axis=0)

    wt, be = _fold_weights(stats_sum.astype(np.float32), bn_gamma, bn_beta, conv_w,
                           conv_b, lin_w, lin_b, linc_w, linc_b)

    if "main" not in _cache:
        _cache["main"] = _build_main_kernel()
    nc2 = _cache["main"]
    in_maps = [{"x": s, "wt": wt, "be": be} for s in shards]
    res2 = run_bass_kernel_spmd(nc2, in_maps, core_ids, trace=TRACE)
    if res2.exec_time_ns:
        exec_ns += res2.exec_time_ns
    LAST_EXEC_NS = exec_ns if exec_ns else None

    outs = []
    for r in res2.results:
        wo = r["wout"].reshape(128, NB, 5).transpose(1, 0, 2).reshape(SHARD, 5)
        outs.append(wo)
    return np.ascontiguousarray(np.concatenate(outs, axis=0))


if __name__ == "__main__":
    pass


# revision 14
# speedup vs baseline: 1.7590x; 1.1042x over previous
"""Trainium2 Bass kernel for nn_ConvNetFullOpti (BN + conv + heads + per-sample
Markowitz QP via PGD).

Strategy (validated against the reference in numpy, maxrel ~5e-4, l2rel ~1e-5):
- Pure data-parallel over 8 cores, 8192 samples each.
- The eigh/matrix-sqrt path is the identity on Q (eigenvalues >> 1e-8), so
  Q = cov directly.
- BN + conv + both linear layers fold into one affine map R^250 -> R^105 whose
  weights depend on the BN batch stats; stats are computed on-device by a small
  first kernel (per-core partial sums), combined on host (the 2KB all-reduce),
  folded into the weight matrix on host, then the main kernel runs.
- Projection onto {sum w = 1, |w| <= 1}: iteration 0 does 12 bisections + 3
  safeguarded Newton steps; subsequent PGD iterations warm-start tau and need
  only 2 safeguarded Newton steps to reproduce the reference's 60-iteration
  bisection to f32 accuracy.

Layout: samples on partitions. Per core: 64 blocks of 128 samples. L2 tiles are
(128 partitions, 64 blocks, ...) so every DVE op covers the whole shard.
"""

import numpy as np

N_CORES = 8
N_TOTAL = 65536
SHARD = N_TOTAL // N_CORES          # 8192
NB = SHARD // 128                   # 64 sample-blocks per core
LOOKBACK, N_ASSETS = 50, 5
NFEAT = 250
NOUT = 105                          # 100 cov-head + 5 ret-head
BN_EPS = 1e-5
PGD_ITERS = 150
NB0_BISECT = 12
NN0_NEWTON = 3
NN_WARM = 2

_cache = {}
TRACE = False           # set True (e.g. by test.py) to capture HW exec times
LAST_EXEC_NS = None     # sum over launches of max-core exec_time_ns when TRACE


def _build_stats_kernel():
    import concourse.bass as bass
    import concourse.tile as tile
    from concourse import bacc, mybir

    dt = mybir.dt
    OP = mybir.AluOpType

    nc = bacc.Bacc("TRN2", target_bir_lowering=False, debug=False)
    x_in = nc.dram_tensor("x", [SHARD, NFEAT], dt.float32, kind="ExternalInput").ap()
    stats_out = nc.dram_tensor("stats", [1, 2 * NFEAT], dt.float32, kind="ExternalOutput").ap()

    with tile.TileContext(nc) as tc:
        with tc.tile_pool(name="io", bufs=4) as io, \
             tc.tile_pool(name="accs", bufs=1) as accs, \
             tc.tile_pool(name="ps", bufs=2, space="PSUM") as ps:
            acc1 = accs.tile([128, NFEAT], dt.float32)
            acc2 = accs.tile([128, NFEAT], dt.float32)
            nc.vector.memset(acc1[:], 0.0)
            nc.vector.memset(acc2[:], 0.0)
            for b in range(NB):
                xb = io.tile([128, NFEAT], dt.float32)
                nc.sync.dma_start(xb[:], x_in[b * 128:(b + 1) * 128, :])
                sq = io.tile([128, NFEAT], dt.float32)
                nc.vector.tensor_tensor(sq[:], xb[:], xb[:], OP.mult)
                nc.vector.tensor_tensor(acc1[:], acc1[:], xb[:], OP.add)
                nc.vector.tensor_tensor(acc2[:], acc2[:], sq[:], OP.add)
            ones = accs.tile([128, 1], dt.float32)
            nc.vector.memset(ones[:], 1.0)
            pr1 = ps.tile([1, NFEAT], dt.float32)
            nc.tensor.matmul(pr1[:], ones[:], acc1[:], start=True, stop=True)
            pr2 = ps.tile([1, NFEAT], dt.float32)
            nc.tensor.matmul(pr2[:], ones[:], acc2[:], start=True, stop=True)
            out_sb = accs.tile([1, 2 * NFEAT], dt.float32)
            nc.vector.tensor_copy(out_sb[:, 0:NFEAT], pr1[:])
            nc.vector.tensor_copy(out_sb[:, NFEAT:2 * NFEAT], pr2[:])
            nc.sync.dma_start(stats_out, out_sb[:])
    nc.compile()
    return nc


def _build_main_kernel():
    import concourse.bass as bass
    import concourse.tile as tile
    from concourse import bacc, mybir, masks

    dt = mybir.dt
    OP = mybir.AluOpType
    AF = mybir.ActivationFunctionType
    AX = mybir.AxisListType
    f32 = dt.float32
    THRESH = float(np.float32(1.0 - 1e-7))

    nc = bacc.Bacc("TRN2", target_bir_lowering=False, debug=False)
    x_in = nc.dram_tensor("x", [SHARD, NFEAT], f32, kind="ExternalInput").ap()
    wt_in = nc.dram_tensor("wt", [NFEAT, NOUT], f32, kind="ExternalInput").ap()
    be_in = nc.dram_tensor("be", [1, NOUT], f32, kind="ExternalInput").ap()
    w_out = nc.dram_tensor("wout", [128, NB * 5], f32, kind="ExternalOutput").ap()

    with tile.TileContext(nc) as tc:
        import contextlib
        ctx = contextlib.ExitStack()
        with ctx:
            consts = ctx.enter_context(tc.tile_pool(name="consts", bufs=1))
            io = ctx.enter_context(tc.tile_pool(name="io", bufs=4))
            xtp = ctx.enter_context(tc.tile_pool(name="xtp", bufs=3))
            big = ctx.enter_context(tc.tile_pool(name="big", bufs=1))
            qp = ctx.enter_context(tc.tile_pool(name="qp", bufs=1))
            ps_t = ctx.enter_context(tc.tile_pool(name="ps_t", bufs=2, space="PSUM"))
            ps_o = ctx.enter_context(tc.tile_pool(name="ps_o", bufs=2, space="PSUM"))

            ident = consts.tile([128, 128], f32)
            masks.make_identity(nc, ident[:])
            ones_row = consts.tile([1, 128], f32)
            nc.vector.memset(ones_row[:], 1.0)
            w0 = consts.tile([125, NOUT], f32)
            nc.sync.dma_start(w0[:], wt_in[0:125, :])
            w1 = consts.tile([125, NOUT], f32)
            nc.sync.dma_start(w1[:], wt_in[125:250, :])
            be = consts.tile([1, NOUT], f32)
            nc.sync.dma_start(be[:], be_in)

            # big persistent L2 tensors; H/NHC are stored (block, asset, time) so
            # the per-(sample,asset) t-broadcast flattens to 3D access patterns
            # (the walrus verifier rejects >3D on TensorScalarPtr ops).
            H = big.tile([128, NB, 5, 20], f32)       # relu(cov-head), (a, t)
            RETS = big.tile([128, NB, 5], f32)
            NHC = big.tile([128, NB, 5, 20], f32)     # -(h - mean_t h), (a, t)
            Q4 = big.tile([128, NB, 5, 5], f32)

            # ---------------- feedforward ----------------
            for b in range(NB):
                xb = io.tile([128, NFEAT], f32)
                nc.sync.dma_start(xb[:], x_in[b * 128:(b + 1) * 128, :])
                xt0 = xtp.tile([125, 128], f32, tag="xt0")
                xt1 = xtp.tile([125, 128], f32, tag="xt1")
                pst0 = ps_t.tile([125, 128], f32, tag="pst0")
                nc.tensor.transpose(pst0[:], xb[:, 0:125], ident[:])
                nc.vector.tensor_copy(xt0[:], pst0[:])
                pst1 = ps_t.tile([125, 128], f32, tag="pst1")
                nc.tensor.transpose(pst1[:], xb[:, 125:250], ident[:])
                nc.scalar.copy(xt1[:], pst1[:])
                po = ps_o.tile([128, NOUT], f32)
                nc.tensor.matmul(po[:], xt0[:], w0[:], start=True, stop=False)
                nc.tensor.matmul(po[:], xt1[:], w1[:], start=False, stop=False)
                nc.tensor.matmul(po[:], ones_row[:], be[:], start=False, stop=True)
                # po columns are (t, a)-ordered; H is (a, t): write via transposed AP
                nc.scalar.activation(
                    H[:, b, :, :].transpose([0, 2, 1]), po[:, 0:100], AF.Relu)
                nc.scalar.activation(RETS[:, b, :], po[:, 100:105], AF.Tanh)

            # ---------------- covariance -> Q ----------------
            HM = qp.tile([128, NB, 5], f32)
            h3 = H[:].rearrange("p b a t -> p (b a) t")
            nc.vector.tensor_reduce(
                HM[:].rearrange("p b a -> p (b a)"), h3, axis=AX.X, op=OP.add)
            # NHC = HM/20 - H  (negated hc; sign cancels in cov)
            hm_b = HM[:].rearrange("p b a -> p (b a)").unsqueeze(2).broadcast_to(
                [128, NB * 5, 20])
            nc.vector.scalar_tensor_tensor(
                NHC[:].rearrange("p b a t -> p (b a) t"), hm_b, 1.0 / 20.0, h3,
                OP.mult, OP.subtract)
            PR20 = qp.tile([128, NB, 20], f32)
            RED = qp.tile([128, NB], f32)
            for i in range(5):
                for j in range(i, 5):
                    nc.vector.tensor_tensor(PR20[:], NHC[:, :, i, :], NHC[:, :, j, :], OP.mult)
                    nc.vector.tensor_reduce(RED[:], PR20[:], axis=AX.X, op=OP.add)
                    nc.vector.tensor_scalar(Q4[:, :, i, j], RED[:], 1.0 / 19.0, None, OP.mult)
                    if i != j:
                        nc.vector.tensor_copy(Q4[:, :, j, i], Q4[:, :, i, j])

            TR = qp.tile([128, NB], f32)
            T1S = qp.tile([128, NB], f32)
            nc.vector.tensor_tensor(TR[:], Q4[:, :, 0, 0], Q4[:, :, 1, 1], OP.add)
            nc.vector.tensor_tensor(TR[:], TR[:], Q4[:, :, 2, 2], OP.add)
            nc.vector.tensor_tensor(TR[:], TR[:], Q4[:, :, 3, 3], OP.add)
            nc.vector.tensor_tensor(TR[:], TR[:], Q4[:, :, 4, 4], OP.add)
            ETA = qp.tile([128, NB], f32)
            nc.vector.tensor_scalar(T1S[:], TR[:], 2.0, 2e-6, OP.mult, OP.add)
            nc.vector.reciprocal(ETA[:], T1S[:])    # eta = 0.5/(tr + 1e-6)

            # ---------------- QP state ----------------
            W = qp.tile([128, NB, 5], f32)
            nc.vector.memset(W[:], 0.2)
            V = qp.tile([128, NB, 5], f32)
            G = qp.tile([128, NB, 5], f32)
            TV = qp.tile([128, NB, 5], f32)
            QW = qp.tile([128, NB, 5], f32)
            T = qp.tile([128, NB, 5], f32)
            C = qp.tile([128, NB, 5], f32)
            AEQ = qp.tile([128, NB, 5], f32)
            NT = qp.tile([128, NB, 5], f32)
            AB = qp.tile([128, NB, 5], f32)
            A2 = qp.tile([128, NB, 5], f32)
            T5 = qp.tile([128, NB, 5], f32)
            TAU = qp.tile([128, NB], f32)
            TAU2 = qp.tile([128, NB], f32)
            LOa = qp.tile([128, NB], f32)
            LOb = qp.tile([128, NB], f32)
            HIa = qp.tile([128, NB], f32)
            HIb = qp.tile([128, NB], f32)
            VMIN = qp.tile([128, NB], f32)
            VMAX = qp.tile([128, NB], f32)
            S = qp.tile([128, NB], f32)
            NA = qp.tile([128, NB], f32)
            PRED = qp.tile([128, NB], dt.int32)   # copy_predicated needs int mask
            NAC = qp.tile([128, NB], f32)
            RNA = qp.tile([128, NB], f32)
            STEP = qp.tile([128, NB], f32)
            CAND = qp.tile([128, NB], f32)
            TT1 = qp.tile([128, NB], f32)
            S2 = qp.tile([128, NB], f32)
            D = qp.tile([128, NB], f32)
            CORR = qp.tile([128, NB], f32)

            def flat(t):
                return t[:].rearrange("p b a -> p (b a)")

            def bcast(t):
                return t[:].unsqueeze(2).broadcast_to([128, NB, 5])

            def eval_t_c_s(tau_t):
                # t = v - tau ; c = clip(t) ; s = sum_a c
                nc.vector.scalar_tensor_tensor(T[:], V[:], 1.0, bcast(tau_t), OP.mult, OP.subtract)
                nc.vector.tensor_scalar(flat(C), flat(T), -1.0, 1.0, OP.max, OP.min)
                nc.vector.tensor_reduce(S[:], C[:], axis=AX.X, op=OP.add)

            def bracket_update(lo, hi, lo2, hi2, tau_t):
                nc.vector.tensor_scalar(PRED[:], S[:], 1.0, None, OP.is_gt)
                nc.vector.select(lo2[:], PRED[:], tau_t[:], lo[:])
                nc.vector.select(hi2[:], PRED[:], hi[:], tau_t[:])

            PR25 = qp.tile([128, NB, 5, 5], f32)
            for it in range(PGD_ITERS):
                # grad + step: Qw_i = sum_j Q[i,j] w_j (4D broadcast + reduce)
                wb4 = W[:].unsqueeze(2).broadcast_to([128, NB, 5, 5])
                nc.vector.tensor_tensor(PR25[:], Q4[:], wb4, OP.mult)
                nc.vector.tensor_reduce(QW[:], PR25[:], axis=AX.X, op=OP.add)
                nc.vector.scalar_tensor_tensor(G[:], QW[:], 2.0, RETS[:], OP.mult, OP.subtract)
                nc.vector.tensor_tensor(TV[:], G[:], bcast(ETA), OP.mult)
                nc.vector.tensor_tensor(V[:], W[:], TV[:], OP.subtract)
                nc.vector.tensor_reduce(VMIN[:], V[:], axis=AX.X, op=OP.min)
                nc.vector.tensor_reduce(VMAX[:], V[:], axis=AX.X, op=OP.max)
                tau, tau2 = TAU, TAU2
                if it == 0:
                    lo, hi, lo2, hi2 = LOa, HIa, LOb, HIb
                    nc.vector.tensor_scalar(lo[:], VMIN[:], -2.0, None, OP.add)
                    nc.vector.tensor_scalar(hi[:], VMAX[:], 2.0, None, OP.add)
                    nc.vector.tensor_tensor(TT1[:], lo[:], hi[:], OP.add)
                    nc.vector.tensor_scalar(tau[:], TT1[:], 0.5, None, OP.mult)
                    for _ in range(NB0_BISECT):
                        eval_t_c_s(tau)
                        bracket_update(lo, hi, lo2, hi2, tau)
                        lo, lo2 = lo2, lo
                        hi, hi2 = hi2, hi
                        nc.vector.tensor_tensor(TT1[:], lo[:], hi[:], OP.add)
                        nc.vector.tensor_scalar(tau[:], TT1[:], 0.5, None, OP.mult)
                    for _ in range(NN0_NEWTON):
                        eval_t_c_s(tau)
                        nc.vector.tensor_tensor(AEQ[:], C[:], T[:], OP.is_equal)
                        nc.vector.tensor_reduce(NA[:], AEQ[:], axis=AX.X, op=OP.add)
                        bracket_update(lo, hi, lo2, hi2, tau)
                        lo, lo2 = lo2, lo
                        hi, hi2 = hi2, hi
                        nc.vector.tensor_scalar(NAC[:], NA[:], 1.0, None, OP.max)
                        nc.vector.reciprocal(RNA[:], NAC[:])
                        nc.vector.scalar_tensor_tensor(STEP[:], S[:], -1.0, RNA[:], OP.add, OP.mult)
                        nc.vector.tensor_tensor(CAND[:], tau[:], STEP[:], OP.add)
                        nc.vector.tensor_tensor(TT1[:], CAND[:], lo[:], OP.max)
                        nc.vector.tensor_tensor(tau2[:], TT1[:], hi[:], OP.min)
                        tau, tau2 = tau2, tau
                else:
                    # clamp warm tau into [vmin-2, vmax+2] via fused stt ops
                    nc.vector.scalar_tensor_tensor(TT1[:], VMIN[:], -2.0, tau[:], OP.add, OP.max)
                    nc.vector.scalar_tensor_tensor(tau2[:], VMAX[:], 2.0, TT1[:], OP.add, OP.min)
                    tau, tau2 = tau2, tau
                # fused eval + finalize + tau update (shares one reciprocal):
                #   w  = c + aeq*(1-s)/max(na,1)
                #   tau' = clamp(tau - (1-s)/max(na,1), vmin-2, vmax+2)
                eval_t_c_s(tau)   # T, C = w0, S = sum w0
                nc.vector.tensor_tensor(AEQ[:], C[:], T[:], OP.is_equal)
                nc.vector.tensor_reduce(NA[:], AEQ[:], axis=AX.X, op=OP.add)
                nc.vector.tensor_scalar(NAC[:], NA[:], 1.0, None, OP.max)
                nc.vector.reciprocal(RNA[:], NAC[:])
                nc.vector.tensor_scalar(D[:], S[:], -1.0, 1.0, OP.mult, OP.add)
                nc.vector.tensor_tensor(CORR[:], D[:], RNA[:], OP.mult)
                nc.vector.scalar_tensor_tensor(T5[:], AEQ[:], 1.0, bcast(CORR), OP.mult, OP.mult)
                nc.vector.tensor_tensor(W[:], C[:], T5[:], OP.add)
                nc.vector.tensor_tensor(CAND[:], tau[:], CORR[:], OP.subtract)
                nc.vector.scalar_tensor_tensor(TT1[:], VMIN[:], -2.0, CAND[:], OP.add, OP.max)
                nc.vector.scalar_tensor_tensor(tau2[:], VMAX[:], 2.0, TT1[:], OP.add, OP.min)
                tau, tau2 = tau2, tau
                if tau is not TAU:
                    TAU, TAU2 = TAU2, TAU

            nc.sync.dma_start(w_out, flat(W))
    nc.compile()
    return nc


def _fold_weights(stats_sum, bn_gamma, bn_beta, conv_w, conv_b, lin_w, lin_b,
                  linc_w, linc_b):
    """Combine global BN stats with the small weights into W_effT (250,105), b_eff."""
    s1 = stats_sum[0]
    s2 = stats_sum[1]
    mu = (s1 / N_TOTAL).astype(np.float32)
    var = (s2 / N_TOTAL - mu * mu).astype(np.float32)
    scale = (bn_gamma / np.sqrt(var + np.float32(BN_EPS))).astype(np.float32)
    shift = (bn_beta - mu * scale).astype(np.float32)

    def conv_mat(c):
        C = np.zeros((120, NFEAT), np.float32)
        for t in range(24):
            for a in range(5):
                for k in range(3):
                    C[5 * t + a, 5 * (2 * t + k) + a] = conv_w[c, 0, k, 0]
        return C

    M = np.vstack([linc_w @ conv_mat(1), lin_w @ conv_mat(0)]).astype(np.float32)
    b0 = np.concatenate([
        linc_b + linc_w.sum(1) * conv_b[1],
        lin_b + lin_w.sum(1) * conv_b[0],
    ]).astype(np.float32)
    W_eff = (M * scale[None, :]).astype(np.float32)
    b_eff = (M @ shift + b0).astype(np.float32)
    return np.ascontiguousarray(W_eff.T), b_eff.reshape(1, NOUT)


def kernel(x, bn_gamma, bn_beta, conv_w, conv_b, lin_w, lin_b, linc_w, linc_b):
    from concourse.bass_utils import run_bass_kernel_spmd

    x = np.ascontiguousarray(np.asarray(x, np.float32).reshape(N_TOTAL, NFEAT))
    bn_gamma = np.asarray(bn_gamma, np.float32)
    bn_beta = np.asarray(bn_beta, np.float32)
    conv_w = np.asarray(conv_w, np.float32)
    conv_b = np.asarray(conv_b, np.float32)
    lin_w = np.asarray(lin_w, np.float32)
    lin_b = np.asarray(lin_b, np.float32)
    linc_w = np.asarray(linc_w, np.float32)
    linc_b = np.asarray(linc_b, np.float32)

    core_ids = list(range(N_CORES))
    shards = [np.ascontiguousarray(x[c * SHARD:(c + 1) * SHARD]) for c in core_ids]

    global LAST_EXEC_NS
    exec_ns = 0
    if "stats" not in _cache:
        _cache["stats"] = _build_stats_kernel()
    nc1 = _cache["stats"]
    res1 = run_bass_kernel_spmd(nc1, [{"x": s} for s in shards], core_ids, trace=TRACE)
    if res1.exec_time_ns:
        exec_ns += res1.exec_time_ns
    stats_sum = np.sum(
        [r["stats"].reshape(2, NFEAT).astype(np.float64) for r in res1.results], axis=0)

    wt, be = _fold_weights(stats_sum.astype(np.float32), bn_gamma, bn_beta, conv_w,
                           conv_b, lin_w, lin_b, linc_w, linc_b)

    if "main" not in _cache:
        _cache["main"] = _build_main_kernel()
    nc2 = _cache["main"]
    in_maps = [{"x": s, "wt": wt, "be": be} for s in shards]
    res2 = run_bass_kernel_spmd(nc2, in_maps, core_ids, trace=TRACE)
    if res2.exec_time_ns:
        exec_ns += res2.exec_time_ns
    LAST_EXEC_NS = exec_ns if exec_ns else None

    outs = []
    for r in res2.results:
        wo = r["wout"].reshape(128, NB, 5).transpose(1, 0, 2).reshape(SHARD, 5)
        outs.append(wo)
    return np.ascontiguousarray(np.concatenate(outs, axis=0))


if __name__ == "__main__":
    pass
